# revision 1
# baseline (speedup 1.0000x reference)
"""Trainium2 Bass kernel for nn_DPModel (DeepPot-SE style GNN message passing).

Strategy: data-parallel over the 1024 atoms (centers) across 8 NeuronCores
(128 centers/core; cores 0-3 handle type-0 centers, 4-7 type-1). Per core:
dense O(N*M) geometry on DVE/ACT, per-center neighbor compaction (cutoff
r<6 selects ~58 of 512 per neighbor-type half) via DVE prefix-scan +
GPSIMD local_scatter into 128 padded slots, embedding MLP + weighted
neighbor reductions on the PE array (fp16 operands, fp32 accumulate),
equivariant outer-product descriptor + per-type fitting net as matmuls.
Each core emits a partial energy scalar; the host sums the 8 partials.
"""
import os
import numpy as np

# ---------------------------------------------------------------------------
# hardcoded problem shapes (self-contained; do not read spec/reference)
N, N1, NTYPES = 1024, 512, 2
NPERCORE, NCORES = 128, 8
RCUT, AXIS, NORM, EPS = 6.0, 16, 64.0, 1e-15
BOXL = 20.0
MAXH = 128          # padded neighbor slots per (center, type-half); data max is 80
W0S, W1S, W2S = 16, 32, 64
FITW = 128

_f32 = np.float32
_f16 = np.float16


def _build_program(debug=False, phases=4):
    import concourse.bass as bass
    import concourse.tile as tile
    from concourse import bacc, mybir

    f32, f16, i32, i16 = (mybir.dt.float32, mybir.dt.float16,
                          mybir.dt.int32, mybir.dt.int16)
    Alu = mybir.AluOpType
    Act = mybir.ActivationFunctionType

    nc = bacc.Bacc("TRN2", target_bir_lowering=False, debug=False,
                   enable_asserts=False)

    def din(name, shape, dt):
        return nc.dram_tensor(name, shape, dt, kind="ExternalInput").ap()

    def dout(name, shape, dt):
        return nc.dram_tensor(name, shape, dt, kind="ExternalOutput").ap()

    # --- DRAM inputs (per-core tensors; host supplies per-core values) ---
    cb_d = din("cb", [3, 128, N], f32)          # coords broadcast to partitions
    cent_d = din("cent", [3, 128, 1], f32)      # this core's center coords
    cidx_d = din("cidx", [128, 1], f32)         # global center indices
    sc_d = din("sc", [128, 4], f32)             # [inv_srstd, -mean/std, inv_xrsr, 0]
    w0_d = din("w0", [2, 16, 128, 128], f16)    # full-K one-hot L1 weights
    w1_d = din("w1", [2, 4, 128, 2 * W1S], f16) # full-K one-hot L2 weights
    w2_d = din("w2", [2, 128, 4 * W2S], f16)    # blkdiag-4 L3 weights
    b0p_d = din("b0p", [2, 128, 1], f32)
    b1p_d = din("b1p", [2, 128, 1], f32)
    b2r_d = din("b2r", [2, 128, 512], f32)      # b2 tiled 8x along free
    id16_d = din("id16", [128, 128], f16)
    id32_d = din("id32", [64, 64], f32)
    gb_d = din("gb", [64, 1], f32)
    bm_d = din("bm", [128, 512], f32)           # Feat block-diag mask
    fw0_d = din("fw0", [64, 16 * FITW], f32)    # fit_W0 as [c, a*128+f]
    fb0_d = din("fb0", [128, 1], f32)
    fw1_d = din("fw1", [FITW, FITW], f32)
    fb1_d = din("fb1", [128, 1], f32)
    fw2_d = din("fw2", [FITW, 1], f32)

    en_d = dout("energy", [1, 1], f32)
    dbg = {}
    if debug:
        dbg['sr'] = dout("dbg_sr", [128, N], f32)
        dbg['mask'] = dout("dbg_mask", [128, N], f32)
        dbg['sel_s'] = dout("dbg_sel_s", [2, 128, MAXH], f16)
        dbg['sel_w0'] = dout("dbg_sel_w0", [2, 128, MAXH], f16)
        dbg['G'] = dout("dbg_G", [64, 512], f32)
        dbg['feat'] = dout("dbg_feat", [64, 2048], f32)

    with tile.TileContext(nc) as tc:
        with (
            tc.tile_pool(name="const", bufs=1) as cpool,
            tc.tile_pool(name="geo", bufs=1) as geo,
            tc.tile_pool(name="cmp", bufs=2) as cmp_,
            tc.tile_pool(name="sel", bufs=1) as selp,
            tc.tile_pool(name="mlp", bufs=3) as mlp,
            tc.tile_pool(name="fin", bufs=1) as fin,
            tc.tile_pool(name="ps", bufs=1, space=bass.MemorySpace.PSUM) as ps,
            tc.tile_pool(name="psg", bufs=1, space=bass.MemorySpace.PSUM) as psg,
        ):
            # ---- load constants ----
            _ldc = [0]

            def load(pool, src, shape, dt):
                _ldc[0] += 1
                t = pool.tile(shape, dt, name=f"ld{_ldc[0]}")
                nc.sync.dma_start(t[:], src)
                return t

            cb = [load(cpool, cb_d[d], [128, N], f32) for d in range(3)]
            cent = [load(cpool, cent_d[d], [128, 1], f32) for d in range(3)]
            cidx = load(cpool, cidx_d, [128, 1], f32)
            sc = load(cpool, sc_d, [128, 4], f32)
            w0t = [[load(cpool, w0_d[j, v], [128, 128], f16) for v in range(16)]
                   for j in range(2)]
            w1t = [[load(cpool, w1_d[j, b], [128, 2 * W1S], f16) for b in range(4)]
                   for j in range(2)]
            w2t = [load(cpool, w2_d[j], [128, 4 * W2S], f16) for j in range(2)]
            b0p = [load(cpool, b0p_d[j], [128, 1], f32) for j in range(2)]
            b1p = [load(cpool, b1p_d[j], [128, 1], f32) for j in range(2)]
            b2r = [load(cpool, b2r_d[j], [128, 512], f32) for j in range(2)]
            id16 = load(cpool, id16_d, [128, 128], f16)
            id32 = load(cpool, id32_d, [64, 64], f32)
            gbias = load(cpool, gb_d, [64, 1], f32)
            bm = load(cpool, bm_d, [128, 512], f32)
            fw0 = load(cpool, fw0_d, [64, 16 * FITW], f32)
            fb0 = load(cpool, fb0_d, [128, 1], f32)
            fw1 = load(cpool, fw1_d, [FITW, FITW], f32)
            fb1 = load(cpool, fb1_d, [128, 1], f32)
            fw2 = load(cpool, fw2_d, [FITW, 1], f32)

            # ---- phase 1: dense geometry, fp32 [128, 1024] planes ----
            TT, TS, STT = nc.vector.tensor_tensor, nc.vector.tensor_scalar, \
                nc.vector.scalar_tensor_tensor

            _gtc = [0]

            def gt(dt=f32):
                _gtc[0] += 1
                return geo.tile([128, N], dt, name=f"geo{_gtc[0]}")

            x = []
            tmp = gt()
            for d in range(3):
                xd = gt()
                TS(xd[:], cb[d][:], cent[d][:, 0:1], None, Alu.subtract)
                # minimum image: x -= 20*(x>=10); x += 20*(x<=-10)
                TS(tmp[:], xd[:], 10.0, None, Alu.is_ge)
                STT(xd[:], tmp[:], -BOXL, xd[:], Alu.mult, Alu.add)
                TS(tmp[:], xd[:], -10.0, None, Alu.is_le)
                STT(xd[:], tmp[:], BOXL, xd[:], Alu.mult, Alu.add)
                x.append(xd)
            r2 = gt()
            nc.scalar.square(r2[:], x[0][:])
            for d in (1, 2):
                nc.scalar.square(tmp[:], x[d][:])
                TT(r2[:], r2[:], tmp[:], Alu.add)
            iota = geo.tile([128, N], f32, name="iota")
            nc.gpsimd.iota(iota[:], [[1, N]], base=0, channel_multiplier=0,
                           allow_small_or_imprecise_dtypes=True)
            diag = gt()
            TS(diag[:], iota[:], cidx[:, 0:1], None, Alu.is_equal)
            TT(r2[:], r2[:], diag[:], Alu.add)
            # r = sqrt(r2) (ACT LUT) then Newton-refine both r and 1/r
            r0 = gt()
            nc.scalar.activation(r0[:], r2[:], Act.Sqrt)
            y0 = gt()
            nc.vector.reciprocal(y0[:], r0[:])
            rr = gt()
            TT(rr[:], r2[:], y0[:], Alu.mult)          # r2/r0
            TT(rr[:], rr[:], r0[:], Alu.add)
            TS(rr[:], rr[:], 0.5, None, Alu.mult)      # r (refined)
            y = gt()
            TT(y[:], rr[:], y0[:], Alu.mult)           # r*y0 ~ 1
            TS(y[:], y[:], -1.0, 2.0, Alu.mult, Alu.add)  # 2 - r*y0
            TT(y[:], y0[:], y[:], Alu.mult)            # y = y0*(2 - r*y0)
            u = gt()
            TS(u[:], rr[:], 1.0 / RCUT, None, Alu.mult)
            mask = gt()
            TS(mask[:], u[:], 1.0, None, Alu.is_lt)
            TT(mask[:], mask[:], diag[:], Alu.subtract)
            pp = gt()
            TS(pp[:], u[:], -6.0, 15.0, Alu.mult, Alu.add)
            TT(pp[:], pp[:], u[:], Alu.mult)
            TS(pp[:], pp[:], -10.0, None, Alu.add)
            u3 = tmp
            TT(u3[:], u[:], u[:], Alu.mult)
            TT(u3[:], u3[:], u[:], Alu.mult)
            TT(pp[:], pp[:], u3[:], Alu.mult)
            sw = pp
            STT(sw[:], pp[:], 1.0, mask[:], Alu.add, Alu.mult)
            sr = gt()
            TT(sr[:], sw[:], y[:], Alu.mult)
            srb = gt()
            TS(srb[:], sr[:], sc[:, 0:1], None, Alu.mult)
            srn = gt()
            TS(srn[:], srb[:], sc[:, 1:2], None, Alu.add)
            rsr = sr
            TT(rsr[:], sr[:], y[:], Alu.mult)
            Rf = y
            TS(Rf[:], rsr[:], sc[:, 2:3], EPS, Alu.mult, Alu.add)
            R = x
            for d in range(3):
                TT(R[d][:], Rf[:], x[d][:], Alu.mult)
            if debug:
                nc.sync.dma_start(dbg['sr'], srb[:])
                nc.sync.dma_start(dbg['mask'], mask[:])

            # ---- phase 2: per-half compaction ----
            if phases < 2:
                ep = fin.tile([1, 1], f32, name="ep_early")
                nc.vector.tensor_copy(ep[:], mask[0:1, 0:1])
                nc.sync.dma_start(en_d, ep[:])
            sel_s, wT = [], []
            for j in (() if phases < 2 else range(2)):
                S = slice(512 * j, 512 * j + 512)
                pos = cmp_.tile([128, 512], f32, tag="pos", name=f"pos{j}")
                nc.vector.tensor_tensor_scan(
                    pos[:], mask[:, S], mask[:, S], 0.0, Alu.add, Alu.bypass)
                sidx = cmp_.tile([128, 512], f32, tag="sidx", name=f"sidx{j}")
                TT(sidx[:], pos[:], mask[:, S], Alu.mult)
                TS(sidx[:], sidx[:], -1.0, None, Alu.add)
                sidx16 = cmp_.tile([128, 512], i16, tag="sidx16", name=f"sidx16_{j}")
                nc.vector.tensor_copy(sidx16[:], sidx[:])
                planes = [srn, srb, R[0], R[1], R[2]]
                sel_j = []
                for pi, pl in enumerate(planes):
                    pl16 = cmp_.tile([128, 512], f16, tag="pl16", name=f"pl16_{j}_{pi}")
                    nc.vector.tensor_copy(pl16[:], pl[:, S])
                    st = selp.tile([128, MAXH], f16, name=f"sel{j}_{pi}")
                    nc.gpsimd.local_scatter(
                        st[:], pl16[:], sidx16[:], channels=128,
                        num_elems=MAXH, num_idxs=512)
                    sel_j.append(st)
                sel_s.append(sel_j[0])
                # transpose the 4 weight planes -> wT_all [128 slots, 4*128 ctr]
                wTj = selp.tile([128, 4 * 128], f16, name=f"wT{j}")
                for k in range(4):
                    tp = ps.tile([128, 128], f32, tag="bD", name=f"tps{j}_{k}")
                    nc.tensor.matmul(tp[:], sel_j[1 + k][:], id16[:])
                    nc.vector.tensor_copy(wTj[:, 128 * k:128 * k + 128], tp[:])
                wT.append(wTj)
                if debug:
                    nc.sync.dma_start(dbg['sel_s'][j], sel_j[0][:])
                    nc.sync.dma_start(dbg['sel_w0'][j], sel_j[1][:])

            # ---- phase 3: MLP + reduce, 128 units per half ----
            # All matmuls use full K=128 (one-hot / blockdiag weights) so PE
            # executes them strictly serially -- concurrent sub-array matmuls
            # into one per-partition PSUM bank SRAM are a hardware fault.
            Gps = [psg.tile([64, 512], f32, tag=f"G{j}", name=f"G{j}")
                   for j in (() if phases < 2.2 else range(2))]
            for j in (() if phases < 2.2 else range(2)):
                for q in range(4):
                    z1 = ps.tile([128, 512], f32, tag="bA" if q % 2 == 0 else "bF",
                                 name=f"z1_{j}_{q}")
                    for g in range(4):
                        nc.tensor.matmul(
                            z1[:, 128 * g:128 * g + 128],
                            w0t[j][4 * q + g][:], sel_s[j][:, :])
                    h1 = mlp.tile([128, 512], f16, tag="h1", name=f"h1_{j}_{q}")
                    nc.scalar.activation(h1[:], z1[:], Act.Tanh, bias=b0p[j][:, 0:1])
                    if phases < 2.4:
                        continue
                    z2 = [ps.tile([128, 512], f32, tag="bB" if h == 0 else "bC",
                                  name=f"z2_{j}_{q}_{h}") for h in range(2)]
                    for g in range(4):
                        for b in range(4):
                            nc.tensor.matmul(
                                z2[b // 2][64 * (b % 2):64 * (b % 2) + 64,
                                           128 * g:128 * g + 128],
                                w1t[j][b][:], h1[:, 128 * g:128 * g + 128],
                                tile_position=(0, 64 * (b % 2)))
                    if phases < 2.6:
                        continue
                    h2 = []
                    for h in range(2):
                        ht = mlp.tile([128, 512], f16, tag=f"h2_{h}",
                                      name=f"h2_{j}_{q}_{h}")
                        nc.scalar.activation(ht[:], z2[h][:], Act.Tanh,
                                             bias=b1p[j][:, 0:1])
                        h2.append(ht)
                    if phases < 2.8:
                        continue
                    # z3 bank: 8 units = both h2 tiles at one g
                    for g in range(4):
                        z3 = ps.tile([128, 512], f32,
                                     tag="bD" if g % 2 == 0 else "bE",
                                     name=f"z3_{j}_{q}_{g}")
                        units = []
                        for h in range(2):
                            nc.tensor.matmul(
                                z3[:, 256 * h:256 * h + 256],
                                h2[h][:, 128 * g:128 * g + 128], w2t[j][:])
                            for p in range(4):
                                uu = 4 * h + 2 * (p // 2) + (p % 2)
                                units.append(32 * g + 8 * q + uu)
                        TT(z3[:], z3[:], b2r[j][:], Alu.add)
                        e8 = mlp.tile([128, 512], f16, tag="e8",
                                      name=f"e8_{j}_{q}_{g}")
                        nc.scalar.activation(e8[:], z3[:], Act.Tanh)
                        if phases >= 3:
                            for w8, u in enumerate(units):
                                nc.tensor.matmul(
                                    Gps[j][0:64, 4 * u:4 * u + 4],
                                    e8[:, 64 * w8:64 * w8 + 64], wT[j][:, u::128])

            if 2.2 <= phases < 4:
                ep3 = fin.tile([1, 1], f32, name="ep_early3")
                nc.vector.tensor_copy(ep3[:], sel_s[0][0:1, 0:1])
                nc.sync.dma_start(en_d, ep3[:])
            # ---- phase 4: G finalize, Feat, fit net ----
            if phases >= 3.2:
                g1s = fin.tile([64, 512], f32)
                nc.vector.tensor_copy(g1s[:], Gps[1][:])
                GS = fin.tile([64, 512], f32)
                TT(GS[:], Gps[0][:], g1s[:], Alu.add)
                TS(GS[:], GS[:], 1.0 / NORM, None, Alu.mult)
                g0 = GS[:, 0::4]
                TS(g0, g0, gbias[:, 0:1], None, Alu.add)
                if debug:
                    nc.sync.dma_start(dbg['G'], GS[:])
                GTS = fin.tile([128, 256], f32)
                for bb in range(4):
                    tp = ps.tile([128, 64], f32, tag="bA", name=f"tpg{bb}")
                    nc.tensor.matmul(tp[:], GS[:, 128 * bb:128 * bb + 128], id32[:])
                    nc.vector.tensor_copy(GTS[:, 64 * bb:64 * bb + 64], tp[:])
                if phases < 3.4:
                    ep4 = fin.tile([1, 1], f32, name="ep4")
                    nc.vector.tensor_copy(ep4[:], GTS[0:1, 0:1])
                    nc.sync.dma_start(en_d, ep4[:])
                featp = [ps.tile([64, 512], f32, tag=["bB", "bC", "bD", "bA"][i],
                                 name=f"fe{i}")
                         for i in (() if phases < 3.4 else range(4))]
                for bb in (() if phases < 3.4 else range(4)):
                    rb = fin.tile([128, 512], f32, name=f"rb{bb}")
                    src_b = GTS[:, None, 64 * bb:64 * bb + 16].broadcast_to(
                        [128, 32, 16])
                    TT(rb[:].rearrange("p (n a) -> p n a", a=16), src_b,
                       bm[:].rearrange("p (n a) -> p n a", a=16), Alu.mult)
                    nc.tensor.matmul(featp[bb][0:64, :],
                                     GTS[:, 64 * bb:64 * bb + 64], rb[:])
                feat = fin.tile([64, 2048], f32)
                for bb in (() if phases < 3.4 else range(4)):
                    nc.vector.tensor_copy(feat[:, 512 * bb:512 * bb + 512], featp[bb][:])
                if debug and phases >= 3.4:
                    nc.sync.dma_start(dbg['feat'], feat[:])
                if 3.4 <= phases < 3.8:
                    ep5 = fin.tile([1, 1], f32, name="ep5")
                    nc.vector.tensor_copy(ep5[:], feat[0:1, 0:1])
                    nc.sync.dma_start(en_d, ep5[:])
                if phases >= 3.8:
                    zf = ps.tile([128, 128], f32, tag="bE")
                    for a in range(16):
                        nc.tensor.matmul(zf[:], fw0[:, FITW * a:FITW * a + FITW],
                                         feat[:, a::16], start=(a == 0), stop=(a == 15))
                    hf = fin.tile([128, 128], f32)
                    nc.scalar.activation(hf[:], zf[:], Act.Tanh, bias=fb0[:, 0:1])
                    zf2 = ps.tile([128, 128], f32, tag="bF")
                    nc.tensor.matmul(zf2[:], fw1[:], hf[:])
                    hf2 = fin.tile([128, 128], f32)
                    nc.scalar.activation(hf2[:], zf2[:], Act.Tanh, bias=fb1[:, 0:1])
                    zrow = ps.tile([1, 128], f32, tag="bE")
                    nc.tensor.matmul(zrow[:], fw2[:], hf2[:])
                    erow = fin.tile([1, 128], f32)
                    nc.vector.tensor_copy(erow[:], zrow[:])
                    eout = fin.tile([1, 1], f32)
                    nc.vector.tensor_reduce(eout[:], erow[:], op=Alu.add,
                                            axis=mybir.AxisListType.X)
                    nc.sync.dma_start(en_d, eout[:])

    nc.compile()
    return nc, dbg


def _host_inputs(inputs):
    """Build the 8 per-core input maps from the full problem inputs."""
    coord = np.asarray(inputs['coord_3N'], _f32)
    srmean = np.asarray(inputs['srmean'], _f32)
    srstd = np.asarray(inputs['srstd'], _f32)
    xrsr = np.asarray(inputs['xrsrstd'], _f32)
    in_maps = []
    for k in range(NCORES):
        t = k // 4
        n0 = NPERCORE * k
        cb = np.broadcast_to(coord[:, None, :], (3, 128, N)).copy()
        cent = coord[:, n0:n0 + 128].reshape(3, 128, 1).copy()
        cidx = np.arange(n0, n0 + 128, dtype=_f32).reshape(128, 1)
        sc = np.zeros((128, 4), _f32)
        sc[:, 0] = 1.0 / srstd[t]
        sc[:, 1] = -srmean[t] / srstd[t]
        sc[:, 2] = 1.0 / xrsr[t]
        w0 = np.zeros((2, 16, 128, 128), _f16)
        w1 = np.zeros((2, 4, 128, 2 * W1S), _f16)
        w2 = np.zeros((2, 128, 4 * W2S), _f16)
        b0p = np.zeros((2, 128, 1), _f32)
        b1p = np.zeros((2, 128, 1), _f32)
        b2r = np.zeros((2, 128, 512), _f32)
        for j in range(2):
            p = 2 * t + j
            W0 = np.asarray(inputs['emb_W0'][p], _f16)[0]   # [16]
            W1 = np.asarray(inputs['emb_W1'][p], _f16)      # [16, 32]
            W2 = np.asarray(inputs['emb_W2'][p], _f16)      # [32, 64]
            for q in range(4):
                for g in range(4):
                    for uu in range(8):
                        w0[j, 4 * q + g, 32 * g + 8 * q + uu,
                           16 * uu:16 * uu + 16] = W0
            for b in range(4):
                for B in range(2):
                    w1[j, b, 16 * (2 * b + B):16 * (2 * b + B) + 16,
                       32 * B:32 * B + 32] = W1
            for pblk in range(4):
                w2[j, 32 * pblk:32 * pblk + 32,
                   W2S * pblk:W2S * pblk + W2S] = W2
            b0 = np.asarray(inputs['emb_b0'][p], _f32)
            b1 = np.asarray(inputs['emb_b1'][p], _f32)
            b2 = np.asarray(inputs['emb_b2'][p], _f32)
            b0p[j, :, 0] = np.tile(b0, 8)
            b1p[j, :, 0] = np.tile(b1, 4)
            b2r[j, :, :] = np.tile(b2, 8)[None, :]
        fit_W0 = np.asarray(inputs['fit_W0'][t], _f32)      # [1024, 128]
        fw0 = np.ascontiguousarray(
            fit_W0.reshape(16, 64, FITW).transpose(1, 0, 2).reshape(64, 16 * FITW))
        in_maps.append({
            "cb": cb, "cent": cent, "cidx": cidx, "sc": sc,
            "w0": w0, "w1": w1, "w2": w2, "b0p": b0p, "b1p": b1p, "b2r": b2r,
            "id16": np.eye(128, dtype=_f16), "id32": np.eye(64, dtype=_f32),
            "gb": np.asarray(inputs['Gbias'], _f32).reshape(64, 1),
            "bm": _feat_blockmask(),
            "fw0": fw0,
            "fb0": np.asarray(inputs['fit_b0'][t], _f32).reshape(128, 1),
            "fw1": np.asarray(inputs['fit_W1'][t], _f32),
            "fb1": np.asarray(inputs['fit_b1'][t], _f32).reshape(128, 1),
            "fw2": np.asarray(inputs['fit_W2'][t], _f32).reshape(FITW, 1),
        })
    return in_maps


def _feat_blockmask():
    bm = np.zeros((128, 512), _f32)
    for p in range(128):
        nl = p // 4
        bm[p, 16 * nl:16 * nl + 16] = 1.0
    return bm


_CACHE = {}


def _get_prog():
    if 'prog' not in _CACHE:
        _CACHE['prog'] = _build_program(debug=False)[0]
    return _CACHE['prog']


def _get_dispatcher():
    """Cached sharded-jit dispatcher (mirrors bass2jax.run_bass_via_pjrt,
    but traces once and keeps the callable + input layout cached)."""
    if 'disp' in _CACHE:
        return _CACHE['disp']
    import jax
    from jax.sharding import Mesh, PartitionSpec
    from jax.experimental.shard_map import shard_map
    from concourse import mybir
    from concourse.bass2jax import (_bass_exec_p, install_neuronx_cc_hook,
                                    partition_id_tensor)
    nc = _get_prog()
    install_neuronx_cc_hook()
    pname = nc.partition_id_tensor.name if nc.partition_id_tensor else None
    in_names, out_names, out_avals, zero_outs = [], [], [], []
    for alloc in nc.m.functions[0].allocations:
        if not isinstance(alloc, mybir.MemoryLocationSet):
            continue
        name = alloc.memorylocations[0].name
        if alloc.kind == "ExternalInput":
            if name != pname:
                in_names.append(name)
        elif alloc.kind == "ExternalOutput":
            shape = tuple(alloc.tensor_shape)
            dtype = mybir.dt.np(alloc.dtype)
            out_names.append(name)
            out_avals.append(jax.core.ShapedArray(shape, dtype))
            zero_outs.append(np.zeros(shape, dtype))
    n_params, n_outs = len(in_names), len(out_names)
    all_in = in_names + out_names + ([pname] if pname else [])

    def _body(*args):
        operands = list(args)
        if pname is not None:
            operands.append(partition_id_tensor())
        return tuple(_bass_exec_p.bind(
            *operands, out_avals=tuple(out_avals), in_names=tuple(all_in),
            out_names=tuple(out_names), lowering_input_output_aliases=(),
            sim_require_finite=True, sim_require_nnan=True, nc=nc))

    devices = jax.devices()[:NCORES]
    mesh = Mesh(np.asarray(devices), ("core",))
    sharded = jax.jit(
        shard_map(_body, mesh=mesh,
                  in_specs=(PartitionSpec("core"),) * (n_params + n_outs),
                  out_specs=(PartitionSpec("core"),) * n_outs,
                  check_rep=False),
        donate_argnums=tuple(range(n_params, n_params + n_outs)),
        keep_unused=True)
    _CACHE['disp'] = (sharded, in_names, out_names, out_avals, zero_outs)
    return _CACHE['disp']


def _run(inputs):
    sharded, in_names, out_names, out_avals, zero_outs = _get_dispatcher()
    in_maps = _host_inputs(inputs)
    concat_in = [np.concatenate([im[n] for im in in_maps], axis=0)
                 for n in in_names]
    concat_zeros = [np.zeros((NCORES * z.shape[0], *z.shape[1:]), z.dtype)
                    for z in zero_outs]
    out_arrs = sharded(*concat_in, *concat_zeros)
    return {name: np.asarray(out_arrs[i]).reshape(NCORES, *out_avals[i].shape)
            for i, name in enumerate(out_names)}


def profile_exec_ns(**inputs):
    """Cost-model (TimelineSim) execution-time estimate in ns; the axon
    client in this container has no NTFF profiling hook."""
    try:
        from concourse.timeline_sim import TimelineSim
        nc = _get_prog()
        return int(TimelineSim(nc, trace=False).simulate())
    except Exception as e:
        print(f"profile pass failed: {e!r}")
        return None


def kernel(**inputs) -> np.ndarray:
    outs = _run(inputs)
    partial = float(outs["energy"].sum())
    # host-side constant: per-atom (fit_b2 + Ebias) summed over all atoms
    fb2 = np.asarray(inputs['fit_b2'], _f32).reshape(-1)
    eb = np.asarray(inputs['Ebias'], _f32).reshape(-1)
    const = N1 * (fb2[0] + eb[0]) + (N - N1) * (fb2[1] + eb[1])
    return np.float32(partial + const)



# revision 16
# speedup vs baseline: 1.3707x; 1.3707x over previous
"""Trainium2 Bass kernel for nn_DPModel (DeepPot-SE style GNN message passing).

Data-parallel over the 1024 atoms across 8 NeuronCores (128 centers/core;
cores 0-3 handle type-0 centers, 4-7 type-1). Per core:

- PE broadcasts neighbor-minus-center deltas (+30) via K=4 f16 hi/lo matmuls
  into PSUM (no coordinate-broadcast DMA).
- Minimum image via one `mod 20` per dim; r^2 from ACT squares (bias -10);
  pair selection mask = (r^2 < 36); self pair and empty slots excluded later
  by a band mask (1e-6 < r^2 < 36) on compacted tiles.
- DVE prefix-scan builds per-(center, type-half) slot indices; GPSIMD
  local_scatter compacts 4 planes (dwx, dwy, dwz, r2) into 128 padded slots.
- Switching function / sr / r*sr computed on compacted [128,128] f16 tiles.
- Embedding MLP on PE (f16, one-hot L1 expansion, block-diag L2/L3), tanh on
  ACT with [128,1024] tiles; b2 bias added by K=1 PE accumulate-matmuls.
- Neighbor reductions G via per-center [128x64]x[128x4] matmuls against
  PE-transposed weight planes (transpose identities pre-scaled by 1/srstd,
  1/xrsrstd). srmean/srstd are folded into W0/b0 on the host.
- Equivariant Feat descriptor + per-type fitting net as matmuls; each core
  emits a partial energy scalar; host sums the 8 partials + bias constant.
"""
import numpy as np

# hardcoded problem shapes (self-contained; do not read spec/reference)
N, N1, NTYPES = 1024, 512, 2
NPERCORE, NCORES = 128, 8
RCUT, AXIS, NORM = 6.0, 16, 64.0
BOXL = 20.0
MAXH = 128
W0S, W1S, W2S = 16, 32, 64
FITW = 128

_f32 = np.float32
_f16 = np.float16

# kmisc f16 column layout
KM_W1 = 0          # 4 blocks (2j+bb) x 128
KM_W2 = 512        # 2 x 256
KM_AID = 1024      # scaled identity (1/srstd) 128
KM_BID = 1152      # scaled identity (1/xrsrstd) 128
KM_BM = 1280       # feat block mask 512
KM_FW1 = 1792      # fit W1 128
KM_FW2 = 1920      # fit W2 1 col (+1 pad)
KM_B2 = 1922       # b2 pattern rows (row 0 only): 2 x 1024
KM_ONE = 3970      # ones row (row 0 only): 128
KM_TOT = 4098

# f32misc column layout
FM_ID32 = 0        # identity 64 (rows 0-63)
FM_GB = 64         # Gbias (rows 0-63)
FM_B0 = 65         # b0' per half: cols 65, 66
FM_B1 = 67         # b1 per half: cols 67, 68
FM_FB0 = 69
FM_FB1 = 70
FM_M10 = 71        # constant -10.0
FM_EPS = 72        # constant 1e-6
FM_TOT = 74


def _build_program(debug=False):
    import concourse.bass as bass
    import concourse.tile as tile
    from concourse import bacc, mybir

    f32, f16, i16 = mybir.dt.float32, mybir.dt.float16, mybir.dt.int16
    Alu = mybir.AluOpType
    Act = mybir.ActivationFunctionType

    nc = bacc.Bacc("TRN2", target_bir_lowering=False, debug=False,
                   enable_asserts=False)

    def din(name, shape, dt):
        return nc.dram_tensor(name, shape, dt, kind="ExternalInput").ap()

    def dout(name, shape, dt):
        return nc.dram_tensor(name, shape, dt, kind="ExternalOutput").ap()

    geo_d = din("geo", [4, 3 * 1152], f16)
    w0a_d = din("w0a", [128, 2048], f16)
    w0b_d = din("w0b", [128, 2048], f16)
    km_d = din("km", [128, KM_TOT], f16)
    fw0_d = din("fw0", [64, 16 * FITW], f16)
    fm_d = din("fm", [128, FM_TOT], f32)
    en_d = dout("energy", [1, 1], f32)
    dbg = {}
    if debug:
        dbg['r2'] = dout("dbg_r2", [128, N], f16)
        dbg['mask'] = dout("dbg_mask", [128, N], f16)
        dbg['gr2'] = dout("dbg_gr2", [2, 128, MAXH], f16)
        dbg['sr'] = dout("dbg_sr", [2, 128, MAXH], f16)
        dbg['G'] = dout("dbg_G", [64, 512], f32)
        dbg['feat'] = dout("dbg_feat", [64, 2048], f16)

    TX = [f"P{i}" for i in range(3)]   # 3 psum slots of [128,1024] f32

    with tile.TileContext(nc) as tc:
        with (
            tc.tile_pool(name="const", bufs=1) as cpool,
            tc.tile_pool(name="dense", bufs=1) as dn,
            tc.tile_pool(name="half", bufs=1) as hf_,
            tc.tile_pool(name="mlp", bufs=2) as mlp,
            tc.tile_pool(name="fin", bufs=1) as fin,
            tc.tile_pool(name="ps", bufs=1, space=bass.MemorySpace.PSUM) as ps,
            tc.tile_pool(name="psg", bufs=1, space=bass.MemorySpace.PSUM) as psg,
        ):
            TT = nc.vector.tensor_tensor
            TS = nc.vector.tensor_scalar
            STT = nc.vector.scalar_tensor_tensor

            # ---- constants (7 DMAs total) ----
            geo = cpool.tile([4, 3 * 1152], f16, name="geo")
            nc.sync.dma_start(geo[:], geo_d)
            w0 = [cpool.tile([128, 2048], f16, name=f"w0_{j}") for j in range(2)]
            nc.sync.dma_start(w0[0][:], w0a_d)
            nc.sync.dma_start(w0[1][:], w0b_d)
            km = cpool.tile([128, KM_TOT], f16, name="km")
            nc.sync.dma_start(km[:], km_d)
            fw0 = cpool.tile([64, 16 * FITW], f16, name="fw0")
            nc.sync.dma_start(fw0[:], fw0_d)
            fm = cpool.tile([128, FM_TOT], f32, name="fm")
            nc.sync.dma_start(fm[:], fm_d)

            # ---- dense phase: delta' = c_j - c_i + 30 in PSUM via PE ----
            dps = []
            for d in range(3):
                t = ps.tile([128, N], f32, tag=TX[d], name=f"dps{d}")
                st = geo[:, 1152 * d + 1024:1152 * d + 1152]   # [4,128]
                for s in range(2):
                    nc.tensor.matmul(t[:, 512 * s:512 * s + 512], st,
                                     geo[:, 1152 * d + 512 * s:1152 * d + 512 * s + 512])
                dps.append(t)

            # min image in f32 (x = delta+30 in (10,50)):
            #   dw = x - 30 - 20*[x>=40] + 20*[x<20]  in [-10,10), f16
            dw = [dn.tile([128, N], f16, name=f"dw{d}") for d in range(3)]
            for d in range(3):
                s1 = dn.tile([128, N], f16, name=f"s1_{d}")
                TS(s1[:], dps[d][:], 40.0, None, Alu.is_ge)
                s2 = dn.tile([128, N], f16, name=f"s2_{d}")
                STT(s2[:], dps[d][:], 20.0, s1[:], Alu.is_lt, Alu.subtract)
                v1 = dn.tile([128, N], f32, name=f"v1_{d}")
                STT(v1[:], s2[:], 20.0, dps[d][:], Alu.mult, Alu.add)
                TS(dw[d][:], v1[:], -30.0, None, Alu.add)

            # r^2 = sum_d dw^2  (ACT squares, DVE adds)
            sq = [dn.tile([128, N], f16, name=f"sq{d}") for d in range(3)]
            for d in range(3):
                nc.scalar.activation(sq[d][:], dw[d][:], Act.Square)
            t01 = dn.tile([128, N], f16, name="t01")
            TT(t01[:], sq[0][:], sq[1][:], Alu.add)
            r2 = dn.tile([128, N], f16, name="r2")
            TT(r2[:], t01[:], sq[2][:], Alu.add)
            mask = dn.tile([128, N], f16, name="mask")
            TS(mask[:], r2[:], 36.0, None, Alu.is_lt)
            if debug:
                nc.sync.dma_start(dbg['r2'], r2[:])
                nc.sync.dma_start(dbg['mask'], mask[:])

            # ---- per-half: compact + switch/sr + weight transposes ----
            sr16, wT = [], []
            for j in range(2):
                S = slice(512 * j, 512 * j + 512)
                pos = hf_.tile([128, 512], f16, name=f"pos{j}")
                nc.vector.tensor_tensor_scan(
                    pos[:], mask[:, S], mask[:, S], 0.0, Alu.add, Alu.bypass)
                sidxf = hf_.tile([128, 512], f16, name=f"sidxf{j}")
                TT(sidxf[:], pos[:], mask[:, S], Alu.mult)
                sidx = hf_.tile([128, 512], i16, name=f"sidx{j}")
                TS(sidx[:], sidxf[:], -1.0, None, Alu.add)

                g_dw = [hf_.tile([128, MAXH], f16, name=f"gdw{j}_{d}")
                        for d in range(3)]
                g_r2 = hf_.tile([128, MAXH], f16, name=f"gr2{j}")
                for d in range(3):
                    nc.gpsimd.local_scatter(
                        g_dw[d][:], dw[d][:, S], sidx[:], channels=128,
                        num_elems=MAXH, num_idxs=512)
                nc.gpsimd.local_scatter(
                    g_r2[:], r2[:, S], sidx[:], channels=128,
                    num_elems=MAXH, num_idxs=512)
                if debug:
                    nc.sync.dma_start(dbg['gr2'][j], g_r2[:])

                # compacted geometry chain ([128,128] f16)
                def ct(nm):
                    return hf_.tile([128, MAXH], f16, name=f"{nm}{j}")
                r0 = ct("r0")
                nc.scalar.activation(r0[:], g_r2[:], Act.Sqrt,
                                     bias=fm[:, FM_EPS:FM_EPS + 1])
                y0 = ct("y0")
                with nc.allow_low_precision(reason="f16 1/r; Newton-refined"):
                    nc.vector.reciprocal(y0[:], r0[:])
                # Newton: r1 = (r2*y0 + r0)/2 ; y = y0*(2 - r1*y0)
                t1 = ct("t1")
                TT(t1[:], g_r2[:], y0[:], Alu.mult)
                t2 = ct("t2")
                TT(t2[:], t1[:], r0[:], Alu.add)       # = 2*r1
                u = ct("u")
                TS(u[:], t2[:], 1.0 / 12.0, None, Alu.mult)
                m0 = ct("m0")
                TT(m0[:], t2[:], y0[:], Alu.mult)      # = 2*r1*y0
                s0 = ct("s0")
                TS(s0[:], m0[:], -0.5, 2.0, Alu.mult, Alu.add)
                y = ct("y")
                TT(y[:], y0[:], s0[:], Alu.mult)
                m1 = ct("m1")
                TS(m1[:], g_r2[:], 36.0, None, Alu.is_lt)
                mc = ct("mc")
                STT(mc[:], g_r2[:], 1e-6, m1[:], Alu.is_gt, Alu.mult)
                p1 = ct("p1")
                TS(p1[:], u[:], -6.0, 15.0, Alu.mult, Alu.add)
                p2 = ct("p2")
                TT(p2[:], p1[:], u[:], Alu.mult)
                u2 = ct("u2")
                TT(u2[:], u[:], u[:], Alu.mult)
                u3 = ct("u3")
                TT(u3[:], u2[:], u[:], Alu.mult)
                p4 = ct("p4")
                STT(p4[:], p2[:], -10.0, u3[:], Alu.add, Alu.mult)
                sw = ct("sw")
                STT(sw[:], p4[:], 1.0, mc[:], Alu.add, Alu.mult)
                sr = ct("sr")
                TT(sr[:], sw[:], y[:], Alu.mult)
                rsr = ct("rsr")
                TT(rsr[:], sr[:], y[:], Alu.mult)
                Rt = []
                for d in range(3):
                    rt = ct(f"Rt{d}")
                    TT(rt[:], g_dw[d][:], rsr[:], Alu.mult)
                    Rt.append(rt)
                sr16.append(sr)
                if debug:
                    nc.sync.dma_start(dbg['sr'][j], sr[:])

                # transpose weight planes -> wT [128 slots, 4*128] (k-major)
                tp = ps.tile([128, 512], f32, tag=TX[j], name=f"tp{j}")
                nc.tensor.matmul(tp[:, 0:128], sr[:], km[:, KM_AID:KM_AID + 128])
                for d in range(3):
                    nc.tensor.matmul(tp[:, 128 * (d + 1):128 * (d + 2)],
                                     Rt[d][:], km[:, KM_BID:KM_BID + 128])
                wTj = hf_.tile([128, 512], f16, name=f"wT{j}")
                nc.vector.tensor_copy(wTj[:], tp[:])
                wT.append(wTj)

            # ---- MLP + G reduction ----
            Gps = [psg.tile([64, 512], f32, tag=f"G{j}", name=f"G{j}")
                   for j in range(2)]
            b0p = [fm[:, FM_B0 + j:FM_B0 + j + 1] for j in range(2)]
            b1p = [fm[:, FM_B1 + j:FM_B1 + j + 1] for j in range(2)]
            zt = 0  # rotating psum slot index for z tiles
            for j in range(2):
                for qp in range(2):
                    z1 = ps.tile([128, 1024], f32, tag=TX[0], name=f"z1_{j}{qp}")
                    for qq in range(2):
                        q = 2 * qp + qq
                        for g in range(4):
                            v = 4 * q + g
                            nc.tensor.matmul(
                                z1[:, 512 * qq + 128 * g:512 * qq + 128 * g + 128],
                                w0[j][:, 128 * v:128 * v + 128], sr16[j][:])
                    h1 = mlp.tile([128, 1024], f16, tag="h1", name=f"h1_{j}{qp}")
                    nc.scalar.activation(h1[:], z1[:], Act.Tanh, bias=b0p[j])
                    z2 = [ps.tile([128, 1024], f32, tag=TX[1 + bb],
                                  name=f"z2_{j}{qp}{bb}") for bb in range(2)]
                    for qq in range(2):
                        for g in range(4):
                            c0 = 512 * qq + 128 * g
                            for bb in range(2):
                                nc.tensor.matmul(
                                    z2[bb][:, c0:c0 + 128],
                                    km[:, KM_W1 + 128 * (2 * j + bb):KM_W1 + 128 * (2 * j + bb) + 128],
                                    h1[:, c0:c0 + 128])
                    h2 = [mlp.tile([128, 1024], f16, tag=f"h2_{bb}",
                                   name=f"h2_{j}{qp}{bb}") for bb in range(2)]
                    for bb in range(2):
                        nc.scalar.activation(h2[bb][:], z2[bb][:], Act.Tanh,
                                             bias=b1p[j])
                    # z3/e8: 8 (qq,g) units; 2 units per [128,1024] psum tile
                    units = [(qq, g) for qq in range(2) for g in range(4)]
                    for t0 in range(4):
                        z3 = ps.tile([128, 1024], f32, tag=TX[zt % 3],
                                     name=f"z3_{j}{qp}{t0}")
                        zt += 1
                        for w in range(2):
                            qq, g = units[2 * t0 + w]
                            c0 = 512 * qq + 128 * g
                            for h in range(2):
                                nc.tensor.matmul(
                                    z3[:, 512 * w + 256 * h:512 * w + 256 * h + 256],
                                    h2[h][:, c0:c0 + 128],
                                    km[:, KM_W2 + 256 * j:KM_W2 + 256 * j + 256],
                                    start=True, stop=False,
                                    skip_group_check=True)
                        for s in range(2):
                            nc.tensor.matmul(
                                z3[:, 512 * s:512 * s + 512],
                                km[0:1, KM_ONE:KM_ONE + 128],
                                km[0:1, KM_B2 + 1024 * j + 512 * s:
                                   KM_B2 + 1024 * j + 512 * s + 512],
                                start=False, stop=True, skip_group_check=True)
                        e8 = mlp.tile([128, 1024], f16, tag="e8",
                                      name=f"e8_{j}{qp}{t0}")
                        nc.scalar.activation(e8[:], z3[:], Act.Tanh)
                        for w in range(2):
                            qq, g = units[2 * t0 + w]
                            q = 2 * qp + qq
                            for h in range(2):
                                for p in range(4):
                                    uu = 4 * h + p
                                    u_ = 32 * g + 8 * q + uu
                                    nc.tensor.matmul(
                                        Gps[j][0:64, 4 * u_:4 * u_ + 4],
                                        e8[:, 512 * w + 256 * h + 64 * p:
                                           512 * w + 256 * h + 64 * p + 64],
                                        wT[j][:, u_::128])

            # ---- G finalize, Feat, fit net ----
            g1s = fin.tile([64, 512], f32, name="g1s")
            TS(g1s[:], Gps[1][:], 1.0 / NORM, None, Alu.mult)
            GS = fin.tile([64, 512], f32, name="GS")
            STT(GS[:], Gps[0][:], 1.0 / NORM, g1s[:], Alu.mult, Alu.add)
            g0 = GS[:, 0::4]
            TS(g0, g0, fm[0:64, FM_GB:FM_GB + 1], None, Alu.add)
            if debug:
                nc.sync.dma_start(dbg['G'], GS[:])
            gtp = psg.tile([128, 256], f32, tag="G0", name="gtp")
            for bb in range(4):
                nc.tensor.matmul(gtp[:, 64 * bb:64 * bb + 64],
                                 GS[:, 128 * bb:128 * bb + 128],
                                 fm[0:64, FM_ID32:FM_ID32 + 64])
            GT = fin.tile([128, 256], f16, name="GT")
            nc.vector.tensor_copy(GT[:], gtp[:])
            feat = fin.tile([64, 2048], f16, name="feat")
            for bb in range(4):
                rb = fin.tile([128, 512], f16, name=f"rb{bb}")
                src_b = GT[:, None, 64 * bb:64 * bb + 16].broadcast_to([128, 32, 16])
                TT(rb[:].rearrange("p (n a) -> p n a", a=16), src_b,
                   km[:, KM_BM:KM_BM + 512].rearrange("p (n a) -> p n a", a=16),
                   Alu.mult)
                fe = ps.tile([64, 512], f32, tag=TX[bb % 3], name=f"fe{bb}")
                nc.tensor.matmul(fe[0:64, :], GT[:, 64 * bb:64 * bb + 64], rb[:])
                nc.vector.tensor_copy(feat[:, 512 * bb:512 * bb + 512], fe[:])
            if debug:
                nc.sync.dma_start(dbg['feat'], feat[:])
            zf = psg.tile([128, 128], f32, tag="G1", name="zf")
            for a in range(16):
                nc.tensor.matmul(zf[:], fw0[:, FITW * a:FITW * a + FITW],
                                 feat[:, a::16], start=(a == 0), stop=(a == 15))
            hf1 = fin.tile([128, 128], f16, name="hf1")
            nc.scalar.activation(hf1[:], zf[:], Act.Tanh,
                                 bias=fm[:, FM_FB0:FM_FB0 + 1])
            zf2 = psg.tile([128, 128], f32, tag="G0", name="zf2")
            nc.tensor.matmul(zf2[:], km[:, KM_FW1:KM_FW1 + 128], hf1[:])
            hf2 = fin.tile([128, 128], f16, name="hf2")
            nc.scalar.activation(hf2[:], zf2[:], Act.Tanh,
                                 bias=fm[:, FM_FB1:FM_FB1 + 1])
            zrow = psg.tile([1, 128], f32, tag="G1", name="zrow")
            nc.tensor.matmul(zrow[:], km[:, KM_FW2:KM_FW2 + 1], hf2[:])
            erow = fin.tile([1, 128], f32, name="erow")
            nc.vector.tensor_copy(erow[:], zrow[:])
            eout = fin.tile([1, 1], f32, name="eout")
            nc.vector.tensor_reduce(eout[:], erow[:], op=Alu.add,
                                    axis=mybir.AxisListType.X)
            nc.sync.dma_start(en_d, eout[:])

    nc.compile()
    return nc, dbg


def _split16(x):
    hi = x.astype(_f16)
    lo = (x.astype(_f32) - hi.astype(_f32)).astype(_f16)
    return hi, lo


def _feat_blockmask():
    bm = np.zeros((128, 512), _f16)
    for p in range(128):
        nl = p // 4
        bm[p, 16 * nl:16 * nl + 16] = 1.0
    return bm


def _host_inputs(inputs):
    """Build the 8 per-core input maps from the full problem inputs."""
    coord = np.asarray(inputs['coord_3N'], _f32)
    srmean = np.asarray(inputs['srmean'], _f32)
    srstd = np.asarray(inputs['srstd'], _f32)
    xrsr = np.asarray(inputs['xrsrstd'], _f32)
    c_hi, c_lo = _split16(coord)          # [3, N] each
    bm = _feat_blockmask()
    in_maps = []
    for k in range(NCORES):
        t = k // 4
        n0 = NPERCORE * k
        cent = coord[:, n0:n0 + 128]       # [3, 128]
        st = (30.0 - cent).astype(_f32)
        st_hi, st_lo = _split16(st)
        geo = np.zeros((4, 3 * 1152), _f16)
        for d in range(3):
            geo[0, 1152 * d:1152 * d + 1024] = c_hi[d]
            geo[1, 1152 * d:1152 * d + 1024] = c_lo[d]
            geo[2, 1152 * d:1152 * d + 1024] = 1.0
            geo[3, 1152 * d:1152 * d + 1024] = 1.0
            geo[0, 1152 * d + 1024:1152 * d + 1152] = 1.0
            geo[1, 1152 * d + 1024:1152 * d + 1152] = 1.0
            geo[2, 1152 * d + 1024:1152 * d + 1152] = st_hi[d]
            geo[3, 1152 * d + 1024:1152 * d + 1152] = st_lo[d]

        w0 = np.zeros((2, 128, 2048), _f16)
        km = np.zeros((128, KM_TOT), _f16)
        fmx = np.zeros((128, FM_TOT), _f32)
        for j in range(2):
            p = 2 * t + j
            W0 = np.asarray(inputs['emb_W0'][p], _f32)[0]   # [16]
            W0p = (W0 / srstd[t]).astype(_f16)
            b0 = np.asarray(inputs['emb_b0'][p], _f32)
            b0p = b0 - (srmean[t] / srstd[t]) * W0
            W1 = np.asarray(inputs['emb_W1'][p], _f16)      # [16, 32]
            W2 = np.asarray(inputs['emb_W2'][p], _f16)      # [32, 64]
            b2 = np.asarray(inputs['emb_b2'][p], _f32)      # [64]
            for q in range(4):
                for g in range(4):
                    v = 4 * q + g
                    for uu in range(8):
                        w0[j, 32 * g + 8 * q + uu, 128 * v + 16 * uu:
                           128 * v + 16 * uu + 16] = W0p
            for bb in range(2):
                for ui in range(4):
                    uu = 4 * bb + ui
                    km[16 * uu:16 * uu + 16,
                       KM_W1 + 128 * (2 * j + bb) + 32 * ui:
                       KM_W1 + 128 * (2 * j + bb) + 32 * ui + 32] = W1
            for pb in range(4):
                km[32 * pb:32 * pb + 32,
                   KM_W2 + 256 * j + 64 * pb:KM_W2 + 256 * j + 64 * pb + 64] = W2
            km[0, KM_B2 + 1024 * j:KM_B2 + 1024 * (j + 1)] = \
                np.tile(b2, 16).astype(_f16)
            fmx[:, FM_B0 + j] = np.tile(b0p, 8)
            fmx[:, FM_B1 + j] = np.tile(np.asarray(inputs['emb_b1'][p], _f32), 4)
        km[:, KM_AID:KM_AID + 128] = (np.eye(128) / srstd[t]).astype(_f16)
        km[:, KM_BID:KM_BID + 128] = (np.eye(128) / xrsr[t]).astype(_f16)
        km[:, KM_BM:KM_BM + 512] = bm
        km[:, KM_FW1:KM_FW1 + 128] = np.asarray(inputs['fit_W1'][t], _f16)
        km[:, KM_FW2] = np.asarray(inputs['fit_W2'][t], _f32).reshape(-1)
        km[0, KM_ONE:KM_ONE + 128] = 1.0
        fit_W0 = np.asarray(inputs['fit_W0'][t], _f32)      # [1024, 128]
        fw0 = np.ascontiguousarray(
            fit_W0.reshape(16, 64, FITW).transpose(1, 0, 2)
            .reshape(64, 16 * FITW)).astype(_f16)
        fmx[0:64, FM_ID32:FM_ID32 + 64] = np.eye(64, dtype=_f32)
        fmx[0:64, FM_GB] = np.asarray(inputs['Gbias'], _f32)
        fmx[:, FM_FB0] = np.asarray(inputs['fit_b0'][t], _f32)
        fmx[:, FM_FB1] = np.asarray(inputs['fit_b1'][t], _f32)
        fmx[:, FM_M10] = -10.0
        fmx[:, FM_EPS] = 1e-6
        in_maps.append({
            "geo": geo, "w0a": w0[0], "w0b": w0[1], "km": km,
            "fw0": fw0, "fm": fmx,
        })
    return in_maps


_CACHE = {}


def _get_prog():
    if 'prog' not in _CACHE:
        _CACHE['prog'] = _build_program(debug=False)[0]
    return _CACHE['prog']


def _get_dispatcher():
    """Cached sharded-jit dispatcher (traces once, keeps callable cached)."""
    if 'disp' in _CACHE:
        return _CACHE['disp']
    import jax
    from jax.sharding import Mesh, PartitionSpec
    from jax.experimental.shard_map import shard_map
    from concourse import mybir
    from concourse.bass2jax import (_bass_exec_p, install_neuronx_cc_hook,
                                    partition_id_tensor)
    nc = _get_prog()
    install_neuronx_cc_hook()
    pname = nc.partition_id_tensor.name if nc.partition_id_tensor else None
    in_names, out_names, out_avals, zero_outs = [], [], [], []
    for alloc in nc.m.functions[0].allocations:
        if not isinstance(alloc, mybir.MemoryLocationSet):
            continue
        name = alloc.memorylocations[0].name
        if alloc.kind == "ExternalInput":
            if name != pname:
                in_names.append(name)
        elif alloc.kind == "ExternalOutput":
            shape = tuple(alloc.tensor_shape)
            dtype = mybir.dt.np(alloc.dtype)
            out_names.append(name)
            out_avals.append(jax.core.ShapedArray(shape, dtype))
            zero_outs.append(np.zeros(shape, dtype))
    n_params, n_outs = len(in_names), len(out_names)
    all_in = in_names + out_names + ([pname] if pname else [])

    def _body(*args):
        operands = list(args)
        if pname is not None:
            operands.append(partition_id_tensor())
        return tuple(_bass_exec_p.bind(
            *operands, out_avals=tuple(out_avals), in_names=tuple(all_in),
            out_names=tuple(out_names), lowering_input_output_aliases=(),
            sim_require_finite=True, sim_require_nnan=True, nc=nc))

    devices = jax.devices()[:NCORES]
    mesh = Mesh(np.asarray(devices), ("core",))
    sharded = jax.jit(
        shard_map(_body, mesh=mesh,
                  in_specs=(PartitionSpec("core"),) * (n_params + n_outs),
                  out_specs=(PartitionSpec("core"),) * n_outs,
                  check_rep=False),
        donate_argnums=tuple(range(n_params, n_params + n_outs)),
        keep_unused=True)
    _CACHE['disp'] = (sharded, in_names, out_names, out_avals, zero_outs)
    return _CACHE['disp']


def _run(inputs):
    sharded, in_names, out_names, out_avals, zero_outs = _get_dispatcher()
    in_maps = _host_inputs(inputs)
    concat_in = [np.concatenate([im[n] for im in in_maps], axis=0)
                 for n in in_names]
    concat_zeros = [np.zeros((NCORES * z.shape[0], *z.shape[1:]), z.dtype)
                    for z in zero_outs]
    out_arrs = sharded(*concat_in, *concat_zeros)
    return {name: np.asarray(out_arrs[i]).reshape(NCORES, *out_avals[i].shape)
            for i, name in enumerate(out_names)}


def profile_exec_ns(**inputs):
    """Cost-model (TimelineSim) execution-time estimate in ns."""
    try:
        from concourse.timeline_sim import TimelineSim
        nc = _get_prog()
        return int(TimelineSim(nc, trace=False).simulate())
    except Exception as e:
        print(f"profile pass failed: {e!r}")
        return None


def kernel(**inputs) -> np.ndarray:
    outs = _run(inputs)
    partial = float(outs["energy"].sum())
    # host-side constant: per-atom (fit_b2 + Ebias) summed over all atoms
    fb2 = np.asarray(inputs['fit_b2'], _f32).reshape(-1)
    eb = np.asarray(inputs['Ebias'], _f32).reshape(-1)
    const = N1 * (fb2[0] + eb[0]) + (N - N1) * (fb2[1] + eb[1])
    return np.float32(partial + const)
